# revision 7
# baseline (speedup 1.0000x reference)
"""DynamicConv Trainium2 kernel, v2 (bf16 I/O + 4-engine tap split).

Problem: x[32,256,64,64] f32. Attention branch (GAP -> FC(64) -> ReLU ->
FC(4) -> softmax) yields per-batch weights attn[b, k] over K=4 depthwise
3x3 kernels; output = sum_k attn[b,k] * depthwise_conv(x, kernel_k).

As in v1, the conv is linear in the kernel taps, so the K kernels are
combined first (w_eff[b,c,tap] = sum_k attn[b,k] conv_w[k,c,tap]) and ONE
depthwise 3x3 conv runs per image with per-(b,c) taps.

Design (cost-model driven; all ops verified legal through the walrus
NEFF compiler -- note the Pool engine cannot run TensorScalarPtr or any
PSUM-operand op, only all-SBUF TensorTensor/TensorCopy + memset/
affine_select/broadcast):
  - x enters and the output leaves in bf16 (host converts); halves the
    dominant DMA traffic. PE diag matmuls are bf16 (same 1 col/cycle
    rate as f32r, but odd PSUM offsets become legal, killing the
    even-rounding fix-up ops).
  - Work is split per 1024-pixel chunk (2 PSUM banks, cpsum bufs=3):
      PE   : 6-7 taps as diagonal matmuls ((0,1) stays on PE only for
             the chunks NOT in DVE01)
      Act  : (0,0) "creator" (activation Copy w/ per-channel scale)
             creates the SBUF partial; also GAP (Copy w/ accum_out),
             ReLU/Exp, and the even diag-slot builds
      DVE  : (-1,-1) as the PSUM-fold tap (scalar_tensor_tensor with
             in1=psum drains PSUM with no separate merge), the (0,1)
             tap as a row-local 3D STT on DVE01 chunks, wrap fix-ups,
             w_eff chain, softmax reciprocal, odd diag-slot builds
      Pool : the output-merge tensor_adds (all-SBUF bf16) + broadcasts
  - Diagonals are built ON-CHIP (identity-mask x per-channel scale),
    4 slots on Act / 3 on DVE -- no DRAM bounce, no DMA sem props on
    the batch-boundary critical path.
  - Emission order keeps every engine queue free of unsatisfiable
    waits: gap(b+1) interleaves with batch b's creators on Act, the
    attention MLP sits late in PE's batch, fix-ups are emitted with
    their chunk (before its matmuls) so they never block earlier folds,
    fold->TT runs with a 2-chunk skew, and the final chunk is split
    into two 512 sub-chunks with private PSUM tiles for a short tail.
  - fc weights are DMA'd before the bulk x(0) halves (fc1_w feeds the
    transposes->MLP->w_eff->diag chain); x(0) loads group-interleaved
    halves feeding a split GAP(0) on DVE+Act.
"""

from contextlib import ExitStack

import numpy as np

B_FULL, C, H, W = 32, 256, 64, 64
K, KS, RED = 4, 3, 4
N_CORES = 8
B_LOC = B_FULL // N_CORES  # 4 images per core
NG = C // 128              # 2 channel groups of 128 partitions
HW = H * W                 # 4096 pixels
QUAD = 1024                # pixels per PSUM chunk (2 banks)
NQ = HW // QUAD            # 2 quads per (image, group)
QROWS = QUAD // W          # 32 image rows per quad

DH = C // RED              # 64 hidden units

# natural tap column order: t = (dy+1)*3 + (dx+1)
# PE diag slot order (matches the two contiguous scatter runs cols 1:4, 6:9)
PE_SLOT = {(-1, 0): 0, (-1, 1): 1, (0, -1): 2, (1, -1): 3, (1, 0): 4, (1, 1): 5,
           (0, 1): 6}
NSLOT = 7
DIAG_PITCH = NSLOT * 128         # per-partition diag row elems (c-major)
DIAG_GRP = 128 * DIAG_PITCH      # per (parity, group) region elems

PE_TAPS_BASE = [
    # chunks 0,1: (1,0) covers [0, HW-64) fully -> start tap
    [(1, 0), (-1, 0), (-1, 1), (0, -1), (0, 1), (1, -1), (1, 1)],
    [(1, 0), (-1, 0), (-1, 1), (0, -1), (0, 1), (1, -1), (1, 1)],
    # chunks 2,3: (-1,0) covers [64, HW) fully -> start tap
    [(-1, 0), (-1, 1), (0, -1), (0, 1), (1, -1), (1, 0), (1, 1)],
    [(-1, 0), (-1, 1), (0, -1), (0, 1), (1, -1), (1, 0), (1, 1)],
]
# chunks whose (0,1) / (1,0) taps run on DVE (into the SBUF part)
DVE01 = {(0, 0), (0, 1), (0, 2), (0, 3), (1, 0)}
DVE10 = set()
DVE0M1 = set()


def pe_taps(g, q):
    taps = list(PE_TAPS_BASE[q])
    if (g, q) in DVE01:
        taps.remove((0, 1))
    if (g, q) in DVE10:
        taps.remove((1, 0))
        if q == 0:
            # (1,0) was the start tap for chunk 0; (1,-1) covers
            # [0, HW-63) which spans chunk 0 fully, so it leads instead
            taps.remove((1, -1))
            taps.insert(0, (1, -1))
    if (g, q) in DVE0M1:
        taps.remove((0, -1))
    return taps


def tap_idx(dy, dx):
    return (dy + 1) * 3 + (dx + 1)


def build_bass():
    import concourse.bacc as bacc
    import concourse.bass as bass
    import concourse.tile as tile
    from concourse import mybir

    f32 = mybir.dt.float32
    bf16 = mybir.dt.bfloat16

    nc = bacc.Bacc("TRN2", target_bir_lowering=False)

    x_d = nc.dram_tensor("x", [B_LOC, C, H, W], bf16, kind="ExternalInput")
    convw_d = nc.dram_tensor("conv_w", [K, C, 1, KS, KS], f32, kind="ExternalInput")
    fc1w_d = nc.dram_tensor("fc1_w", [DH, C], f32, kind="ExternalInput")
    fc1b_d = nc.dram_tensor("fc1_b", [DH], f32, kind="ExternalInput")
    fc2w_d = nc.dram_tensor("fc2_w", [K, DH], f32, kind="ExternalInput")
    fc2b_d = nc.dram_tensor("fc2_b", [K], f32, kind="ExternalInput")
    out_d = nc.dram_tensor("out", [B_LOC, C, H, W], bf16, kind="ExternalOutput")

    with tile.TileContext(nc) as tc, ExitStack() as ctx:
        singles = ctx.enter_context(tc.tile_pool(name="singles", bufs=1))
        xin = ctx.enter_context(tc.tile_pool(name="xin", bufs=6))
        parts = ctx.enter_context(tc.tile_pool(name="parts", bufs=8))
        accs = ctx.enter_context(tc.tile_pool(name="accs", bufs=4))
        outs = ctx.enter_context(tc.tile_pool(name="outs", bufs=4))
        diags = ctx.enter_context(tc.tile_pool(name="diags", bufs=4))
        smalls = ctx.enter_context(tc.tile_pool(name="smalls", bufs=3))
        cpsum = ctx.enter_context(tc.tile_pool(name="cpsum", bufs=3, space="PSUM"))
        mpsum = ctx.enter_context(tc.tile_pool(name="mpsum", bufs=1, space="PSUM"))
        wpsum = ctx.enter_context(tc.tile_pool(name="wpsum", bufs=1, space="PSUM"))

        # ---- identity mask for on-chip diagonal builds ---------------------
        # diag(w) = ident128 * w (per-partition scale) on Act/Pool: no DRAM
        # bounce, no scatter/load DMAs, no 900ns DMA sem props on the
        # batch-boundary critical path.
        onesb = singles.tile([128, 128], bf16, tag="onesb")
        nc.gpsimd.memset(onesb[:], 1.0)
        identb = singles.tile([128, 128], bf16, tag="identb")
        nc.gpsimd.affine_select(
            out=identb[:], in_=onesb[:], pattern=[[-1, 128]],
            compare_op=mybir.AluOpType.is_equal, fill=0.0,
            base=0, channel_multiplier=1)

        x_tiles = {}

        def emit_load(b, halves=False):
            x_t = []
            for g in range(NG):
                t = xin.tile([128, HW], bf16, tag="x", name=f"x_{b}_{g}")
                src = x_d[b, g * 128:(g + 1) * 128, :, :].rearrange(
                    "p h w -> p (h w)")
                if halves:
                    for q in range(2):
                        lo = q * (HW // 2)
                        nc.sync.dma_start(out=t[:, lo:lo + HW // 2],
                                          in_=src[:, lo:lo + HW // 2])
                else:
                    nc.sync.dma_start(out=t[:], in_=src)
                x_t.append(t)
            return x_t

        # fc weights first: fc1_w feeds the PE transposes -> MLP -> w_eff
        # -> diag chain; it must not sit behind 5.8us of x halves in the
        # single DMA FIFO.
        fc1w_sb = singles.tile([DH, C], f32, tag="fc1w_sb")
        nc.sync.dma_start(out=fc1w_sb[:], in_=fc1w_d[:])
        fc2wT = singles.tile([DH + 1, K], f32, tag="fc2wT")
        nc.sync.dma_start(
            out=fc2wT[:DH, :],
            in_=bass.AP(tensor=fc2w_d, offset=0, ap=[[1, DH], [DH, K]]),
        )
        nc.sync.dma_start(out=fc2wT[DH:DH + 1, :],
                          in_=bass.AP(tensor=fc2b_d, offset=0,
                                      ap=[[K, 1], [1, K]]))
        fc1b = singles.tile([DH, 1], f32, tag="fc1b")
        nc.sync.dma_start(out=fc1b[:], in_=fc1b_d[:].unsqueeze(1))

        xt0 = [xin.tile([128, HW], bf16, tag="x", name=f"x_0_{g}")
               for g in range(NG)]
        for q in range(2):
            lo = q * (HW // 2)
            for g in range(NG):
                src_ = x_d[0, g * 128:(g + 1) * 128, :, :].rearrange(
                    "p h w -> p (h w)")
                nc.sync.dma_start(out=xt0[g][:, lo:lo + HW // 2],
                                  in_=src_[:, lo:lo + HW // 2])
        x_tiles[0] = xt0

        convw_sb = [[None] * K for _ in range(NG)]
        for g in range(NG):
            for k in range(K):
                t = singles.tile([128, KS * KS], f32, tag=f"cw{g}_{k}")
                src = bass.AP(tensor=convw_d,
                              offset=k * C * KS * KS + g * 128 * KS * KS,
                              ap=[[KS * KS, 128], [1, KS * KS]])
                nc.sync.dma_start(out=t[:], in_=src)
                convw_sb[g][k] = t

        # fc1_wT[g]: [c in group (partitions), m] = fc1_w[m, c] / HW, built
        # by PE transpose of a contiguous load (strided 4B gathers are slow).
        ones64 = singles.tile([DH, DH], f32, tag="ones64")
        nc.gpsimd.memset(ones64[:], 1.0)
        ident64 = singles.tile([DH, DH], f32, tag="ident64")
        nc.gpsimd.affine_select(
            out=ident64[:], in_=ones64[:], pattern=[[-1, DH]],
            compare_op=mybir.AluOpType.is_equal, fill=0.0,
            base=0, channel_multiplier=1)

        fc1wT = []
        for g in range(NG):
            tps = mpsum.tile([128, DH], f32, tag="mlp")
            nc.tensor.transpose(tps[:], fc1w_sb[:, g * 128:(g + 1) * 128],
                                ident64[:])
            t = singles.tile([128, DH], f32, tag=f"fc1wT{g}")
            nc.scalar.mul(t[:], tps[:], 1.0 / HW)
            fc1wT.append(t)

        gapscr = singles.tile([128, HW], bf16, tag="gapscr")

        def emit_gap_group(b, x_t, gsum, g):
            """Act: per-channel spatial sum of group g into gsum[:, g]."""
            nc.scalar.activation(gapscr[:], x_t[g][:],
                                 mybir.ActivationFunctionType.Copy,
                                 bias=0.0, scale=1.0,
                                 accum_out=gsum[:, g:g + 1])

        def emit_attn_mlp(b, gsum):
            """PE: the tiny attention MLP (with Act ReLU between layers)."""
            h_ps = mpsum.tile([DH, 1], f32, tag="mlp")
            for g in range(NG):
                nc.tensor.matmul(h_ps[:], fc1wT[g][:], gsum[:, g:g + 1],
                                 start=(g == 0), stop=(g == NG - 1))
            h_sb = smalls.tile([DH + 1, 1], f32, tag="h_sb")
            nc.scalar.activation(h_sb[:DH], h_ps[:],
                                 mybir.ActivationFunctionType.Relu,
                                 bias=fc1b[:], scale=1.0)
            nc.vector.memset(h_sb[DH:DH + 1, :], 1.0)
            a_ps = mpsum.tile([1, K], f32, tag="mlp")
            nc.tensor.matmul(a_ps[:], h_sb[:], fc2wT[:], start=True, stop=True)
            return a_ps

        def emit_attn_tail(b, a_ps):
            """Act: softmax pieces; Pool: w_eff build; DMA: diag scatter+load.

            Runs on Act/Pool/SP only so neither PE matmuls nor DVE folds
            ever queue behind it.
            """
            expv = smalls.tile([1, K], f32, tag="expv")
            ssum = smalls.tile([1, 1], f32, tag="ssum")
            nc.scalar.activation(expv[:], a_ps[:],
                                 mybir.ActivationFunctionType.Exp,
                                 bias=0.0, scale=1.0, accum_out=ssum[:])
            # 1/ssum on DVE (tiny); emitted here it sits between folds in
            # DVE program order, but the fold->TT skew absorbs the wait.
            rsum = smalls.tile([1, 1], f32, tag="rsum")
            nc.vector.reciprocal(rsum[:], ssum[:])
            e_bc = smalls.tile([128, K], f32, tag="e_bc")
            nc.gpsimd.partition_broadcast(e_bc[:], expv[:])
            r_bc = smalls.tile([128, 1], f32, tag="r_bc")
            nc.gpsimd.partition_broadcast(r_bc[:], rsum[:])

            weff, negw, diag_sb = [], [], []
            for g in range(NG):
                wt = smalls.tile([128, KS * KS], f32, tag=f"weff{g}")
                nc.vector.tensor_scalar_mul(wt[:], convw_sb[g][0][:],
                                            e_bc[:, 0:1])
                for k in range(1, K):
                    nc.vector.scalar_tensor_tensor(
                        out=wt[:], in0=convw_sb[g][k][:],
                        scalar=e_bc[:, k:k + 1], in1=wt[:],
                        op0=mybir.AluOpType.mult, op1=mybir.AluOpType.add)
                nc.vector.tensor_scalar_mul(wt[:], wt[:], r_bc[:])
                weff.append(wt)
                nt = smalls.tile([128, KS * KS], f32, tag=f"negw{g}")
                nc.scalar.mul(nt[:], wt[:], -1.0)
                negw.append(nt)

                # build the 6 PE diag slots on-chip: diag = ident * w_eff[col]
                # (3 slots on Act, 3 on Pool; disjoint writes run in parallel)
                dt_ = diags.tile([128, DIAG_PITCH], bf16, tag="diag",
                                 name=f"diag_{b}_{g}")
                for (dy, dx), sl in PE_SLOT.items():
                    col = tap_idx(dy, dx)
                    dst = dt_[:, sl * 128:(sl + 1) * 128]
                    if sl % 2 == 0:
                        nc.scalar.activation(dst, identb[:],
                                             mybir.ActivationFunctionType.Copy,
                                             bias=0.0,
                                             scale=wt[:, col:col + 1])
                    else:
                        nc.vector.tensor_scalar_mul(dst, identb[:],
                                                    wt[:, col:col + 1])
                diag_sb.append(dt_)
            return weff, negw, diag_sb

        def emit_creator(b, g, q, x_t, weff):
            """Act: part = w_(0,0) * x over the quad (bf16)."""
            q0 = q * QUAD
            part = parts.tile([128, QUAD], bf16, tag="part",
                              name=f"part_{b}_{g}_{q}")
            nc.scalar.activation(part[:], x_t[g][:, q0:q0 + QUAD],
                                 mybir.ActivationFunctionType.Copy,
                                 bias=0.0, scale=weff[g][:, 4:5])
            return part

        def emit_fixups(b, g, q, x_t, part, negw, weff_all):
            """DVE: the taps moved off PE for this chunk (into the SBUF
            part), then the row-wrap subtractions for the remaining flat
            PE taps with dx != 0."""
            q0 = q * QUAD
            x3 = x_t[g][:].rearrange("p (h w) -> p h w", w=W)
            p3 = part[:].rearrange("p (h w) -> p h w", w=W)
            r0 = q * QROWS
            if (g, q) in DVE01:
                # (0,1) tap: row-local 3D AP, no wrap, no fixup needed
                nc.vector.scalar_tensor_tensor(
                    out=p3[:, 0:QROWS, 0:W - 1],
                    in0=x3[:, r0:r0 + QROWS, 1:W],
                    scalar=weff_all[g][:, 5:6], in1=p3[:, 0:QROWS, 0:W - 1],
                    op0=mybir.AluOpType.mult, op1=mybir.AluOpType.add)
            if (g, q) in DVE10:
                n = min(QUAD, HW - W - q0)
                nc.vector.scalar_tensor_tensor(
                    out=part[:, 0:n], in0=x_t[g][:, q0 + W:q0 + W + n],
                    scalar=weff_all[g][:, 7:8], in1=part[:, 0:n],
                    op0=mybir.AluOpType.mult, op1=mybir.AluOpType.add)
            if (g, q) in DVE0M1:
                nc.vector.scalar_tensor_tensor(
                    out=p3[:, 0:QROWS, 1:W],
                    in0=x3[:, r0:r0 + QROWS, 0:W - 1],
                    scalar=weff_all[g][:, 3:4], in1=p3[:, 0:QROWS, 1:W],
                    op0=mybir.AluOpType.mult, op1=mybir.AluOpType.add)
            eng = nc.vector
            for (dy, dx) in pe_taps(g, q):
                if dx == 0:
                    continue
                ti = tap_idx(dy, dx)
                S = W * dy + dx
                t0 = max(0, -S)
                t1 = HW - max(0, S)
                i0 = max(q0, t0)
                i1 = min(q0 + QUAD, t1)
                e = 0 if dx < 0 else W - 1
                s = W - 1 - e
                d = dy + dx
                h0 = -(-(i0 - e) // W)
                h1 = (i1 - 1 - e) // W + 1
                eng.scalar_tensor_tensor(
                    out=p3[:, h0 - q * QROWS:h1 - q * QROWS, e:e + 1],
                    in0=x3[:, h0 + d:h1 + d, s:s + 1],
                    scalar=negw[g][:, ti:ti + 1],
                    in1=p3[:, h0 - q * QROWS:h1 - q * QROWS, e:e + 1],
                    op0=mybir.AluOpType.mult, op1=mybir.AluOpType.add)

        def emit_pe_quad(b, g, q, x_t, diag_sb, ps):
            """PE: diag matmuls for this quad's taps, accumulated in PSUM."""
            q0 = q * QUAD
            xr = x_t[g][:]
            taps = pe_taps(g, q)
            for i, (dy, dx) in enumerate(taps):
                S = W * dy + dx
                t0 = max(0, -S)
                t1 = HW - max(0, S)
                sl = PE_SLOT[(dy, dx)]
                for j in range(QUAD // 512):
                    b0 = q0 + j * 512
                    i0 = max(b0, t0)
                    i1 = min(b0 + 512, t1)
                    nc.tensor.matmul(
                        ps[:, i0 - q0:i1 - q0],
                        diag_sb[g][:, sl * 128:(sl + 1) * 128],
                        xr[:, i0 + S:i1 + S],
                        start=(i == 0), stop=(i == len(taps) - 1),
                        skip_group_check=True)

        def emit_fold(b, g, q, x_t, ps, weff, negw):
            """DVE: fold tap (-1,-1) drains PSUM into acc and fix its col-0
            wrap. Emitted ahead of the previous chunk's TT so PSUM turnover
            never waits behind a part-chain dependency."""
            q0 = q * QUAD
            x3 = x_t[g][:].rearrange("p (h w) -> p h w", w=W)
            acc = accs.tile([128, QUAD], bf16, tag="acc",
                            name=f"acc_{b}_{g}_{q}")
            a3 = acc[:].rearrange("p (h w) -> p h w", w=W)
            S = -W - 1
            if q == 0:
                # rows 0/1 cols 0..64 have no (-1,-1) contribution
                nc.vector.tensor_copy(acc[:, 0:-S], ps[:, 0:-S])
                nc.vector.scalar_tensor_tensor(
                    out=acc[:, -S:QUAD], in0=x_t[g][:, 0:QUAD + S],
                    scalar=weff[g][:, 0:1], in1=ps[:, -S:QUAD],
                    op0=mybir.AluOpType.mult, op1=mybir.AluOpType.add)
                rf0 = 2
            else:
                nc.vector.scalar_tensor_tensor(
                    out=acc[:], in0=x_t[g][:, q0 + S:q0 + QUAD + S],
                    scalar=weff[g][:, 0:1], in1=ps[:],
                    op0=mybir.AluOpType.mult, op1=mybir.AluOpType.add)
                rf0 = 0
            # col-0 wrap fix: acc[r,0] -= w * x[r-2, 63]
            r0 = q * QROWS
            nc.vector.scalar_tensor_tensor(
                out=a3[:, rf0:QROWS, 0:1],
                in0=x3[:, r0 + rf0 - 2:r0 + QROWS - 2, W - 1:W],
                scalar=negw[g][:, 0:1], in1=a3[:, rf0:QROWS, 0:1],
                op0=mybir.AluOpType.mult, op1=mybir.AluOpType.add)
            return acc

        def emit_tt(b, g, q, acc, part, pieces=1):
            """All-bf16 SBUF tensor_tensor writes the output tile. Runs on
            the otherwise-idle Pool engine (legal for SBUF TensorTensor);
            the final batch's last chunks stay on DVE for a short tail."""
            q0 = q * QUAD
            eng = nc.vector if (b == B_LOC - 1 and g == 1 and q >= 2) \
                else nc.gpsimd
            out_t = outs.tile([128, QUAD], bf16, tag="out",
                              name=f"out_{b}_{g}_{q}")
            dst = out_d[b, g * 128:(g + 1) * 128, :, :].rearrange(
                "p h w -> p (h w)")[:, q0:q0 + QUAD]
            span = QUAD // pieces
            for p_ in range(pieces):
                lo = p_ * span
                eng.tensor_add(out_t[:, lo:lo + span],
                               acc[:, lo:lo + span],
                               part[:, lo:lo + span])
                nc.sync.dma_start(out=dst[:, lo:lo + span],
                                  in_=out_t[:, lo:lo + span])

        onesw = singles.tile([128, 512], bf16, tag="onesw")
        nc.gpsimd.memset(onesw[:], 1.0)

        def emit_warmup(n512, n64):
            wp = wpsum.tile([128, 512], f32, tag="warm")
            for _ in range(n512):
                nc.tensor.matmul(wp[:], identb[:], onesw[:],
                                 start=True, stop=True, skip_group_check=True)
            for _ in range(n64):
                nc.tensor.matmul(wp[:, 0:64], identb[:], onesw[:, 0:64],
                                 start=True, stop=True, skip_group_check=True)


        def emit_last_chunk_split(b, x_t, weff, negw, part, diag_sb):
            """Final chunk (g=1,q=3) as two 512 halves so the last fold and
            store overlap the preceding matmuls: short kernel tail."""
            g, q = 1, 3
            emit_fixups(b, g, q, x_t, part, negw, weff)
            q0 = q * QUAD
            xr = x_t[g][:]
            x3 = xr.rearrange("p (h w) -> p h w", w=W)
            taps = pe_taps(g, q)
            dst = out_d[b, g * 128:(g + 1) * 128, :, :].rearrange(
                "p h w -> p (h w)")
            S0 = -W - 1
            for h_ in range(2):
                b0 = q0 + h_ * 512
                ps = cpsum.tile([128, 512], mybir.dt.float32, tag="cps",
                                name=f"cps_{b}_{g}_{q}_{h_}")
                for i, (dy, dx) in enumerate(taps):
                    S = W * dy + dx
                    t0 = max(0, -S)
                    t1 = HW - max(0, S)
                    i0 = max(b0, t0)
                    i1 = min(b0 + 512, t1)
                    nc.tensor.matmul(
                        ps[:, i0 - b0:i1 - b0],
                        diag_sb[g][:, PE_SLOT[(dy, dx)] * 128:
                                    (PE_SLOT[(dy, dx)] + 1) * 128],
                        xr[:, i0 + S:i1 + S],
                        start=(i == 0), stop=(i == len(taps) - 1),
                        skip_group_check=True)
                out_t = outs.tile([128, 512], bf16, tag="out",
                                  name=f"out_{b}_{g}_{q}_{h_}")
                nc.vector.scalar_tensor_tensor(
                    out=out_t[:], in0=xr[:, b0 + S0:b0 + 512 + S0],
                    scalar=weff[g][:, 0:1], in1=ps[:],
                    op0=mybir.AluOpType.mult, op1=mybir.AluOpType.add)
                o3 = out_t[:].rearrange("p (h w) -> p h w", w=W)
                r0 = b0 // W
                nc.vector.scalar_tensor_tensor(
                    out=o3[:, 0:512 // W, 0:1],
                    in0=x3[:, r0 - 2:r0 + 512 // W - 2, W - 1:W],
                    scalar=negw[g][:, 0:1], in1=o3[:, 0:512 // W, 0:1],
                    op0=mybir.AluOpType.mult, op1=mybir.AluOpType.add)
                nc.vector.tensor_add(out_t[:], out_t[:],
                                     part[:, b0 - q0:b0 - q0 + 512])
                nc.sync.dma_start(out=dst[:, b0:b0 + 512], in_=out_t[:])

        # ---- prologue: batch 0's attention chain --------------------------
        # GAP(0) in half-tile pieces, each as soon as its DMA lands
        gsum0 = smalls.tile([128, NG], f32, tag="gsum", name="gsum_0")
        ghalf = smalls.tile([128, 4], f32, tag="ghalf")
        for q in range(2):
            lo = q * (HW // 2)
            nc.vector.tensor_reduce(out=ghalf[:, q:q + 1],
                                    in_=x_tiles[0][0][:, lo:lo + HW // 2],
                                    axis=mybir.AxisListType.X,
                                    op=mybir.AluOpType.add)
            nc.scalar.activation(gapscr[:, lo:lo + HW // 2],
                                 x_tiles[0][1][:, lo:lo + HW // 2],
                                 mybir.ActivationFunctionType.Copy,
                                 bias=0.0, scale=1.0,
                                 accum_out=ghalf[:, 2 + q:3 + q])
        nc.vector.tensor_add(gsum0[:, 0:1], ghalf[:, 0:1], ghalf[:, 1:2])
        nc.vector.tensor_add(gsum0[:, 1:2], ghalf[:, 2:3], ghalf[:, 3:4])
        emit_warmup(0, 0)
        a_ps0 = emit_attn_mlp(0, gsum0)
        stages = {0: emit_attn_tail(0, a_ps0)}
        emit_warmup(0, 0)
        if B_LOC > 1:
            x_tiles[1] = emit_load(1)

        # ---- main pipeline, one batch ahead -------------------------------
        gsums = {}
        for b in range(B_LOC):
            x_t = x_tiles.pop(b)
            weff, negw, diag_sb = stages.pop(b)
            nxt = b + 1 < B_LOC

            # Act: creators for batch b interleaved with batch b+1's GAP
            # (gap(g0) after the first creator, gap(g1) after g0's creators,
            # so gsum(b+1) is ready before PE's mid-batch MLP while the
            # early part chains are not starved)
            if nxt:
                gsums[b + 1] = smalls.tile([128, NG], f32, tag="gsum",
                                           name=f"gsum_{b + 1}")
            part_q = {}
            part_q[(0, 0)] = emit_creator(b, 0, 0, x_t, weff)
            if nxt:
                emit_gap_group(b + 1, x_tiles[b + 1], gsums[b + 1], 0)
            for q in range(1, NQ):
                part_q[(0, q)] = emit_creator(b, 0, q, x_t, weff)
            if nxt:
                emit_gap_group(b + 1, x_tiles[b + 1], gsums[b + 1], 1)
            for q in range(NQ):
                part_q[(1, q)] = emit_creator(b, 1, q, x_t, weff)

            # conv chunks with a fold->TT skew: fixups (incl the DVE (0,1)
            # tap) are emitted just before each chunk's matmuls so they
            # never block earlier folds in DVE program order.
            pending = []
            skew = 2

            def conv_chunk(g, q, pieces=1):
                emit_fixups(b, g, q, x_t, part_q[(g, q)], negw, weff)
                ps = cpsum.tile([128, QUAD], mybir.dt.float32, tag="cps",
                                name=f"cps_{b}_{g}_{q}")
                emit_pe_quad(b, g, q, x_t, diag_sb, ps)
                acc = emit_fold(b, g, q, x_t, ps, weff, negw)
                pending.append((g, q, acc, part_q[(g, q)], pieces))
                if len(pending) > skew:
                    emit_tt(b, *pending.pop(0))

            # PE: group 0 conv chunks + first half of group 1
            for (g, q) in [(0, 0), (0, 1), (0, 2), (0, 3), (1, 0), (1, 1)]:
                conv_chunk(g, q)

            # attention for b+1 (PE MLP here, late in the batch)
            if nxt:
                a_ps = emit_attn_mlp(b + 1, gsums.pop(b + 1))
                stages[b + 1] = emit_attn_tail(b + 1, a_ps)

            if b + 2 < B_LOC:
                x_tiles[b + 2] = emit_load(b + 2)

            # PE: group 1's last two chunks; on the final batch, drain the
            # pending TTs before the last chunk's fold so the kernel tail is
            # just fold -> TT -> store
            last_b = b == B_LOC - 1
            conv_chunk(1, 2, pieces=2 if last_b else 1)
            if last_b:
                while len(pending) > 1:
                    emit_tt(b, *pending.pop(0))
                emit_last_chunk_split(b, x_t, weff, negw, part_q[(1, 3)],
                                      diag_sb)
                while pending:
                    emit_tt(b, *pending.pop(0))
            else:
                conv_chunk(1, 3)
            if not last_b:
                while pending:
                    emit_tt(b, *pending.pop(0))

    nc.compile()
    return nc


_COMPILED = None
LAST_RESULTS = None


def kernel(**inputs):
    global _COMPILED
    import ml_dtypes
    from concourse.bass_utils import run_bass_kernel_spmd

    if _COMPILED is None:
        _COMPILED = build_bass()
    nc = _COMPILED

    x = np.asarray(inputs["x"], dtype=np.float32).astype(ml_dtypes.bfloat16)
    rep = {k: np.ascontiguousarray(v, dtype=np.float32)
           for k, v in inputs.items() if k != "x"}

    in_maps = []
    for i in range(N_CORES):
        m = {"x": np.ascontiguousarray(x[i * B_LOC:(i + 1) * B_LOC])}
        m.update(rep)
        in_maps.append(m)

    last_exc = None
    for attempt in range(3):
        try:
            res = run_bass_kernel_spmd(nc, in_maps,
                                       core_ids=list(range(N_CORES)))
            break
        except Exception as e:  # noqa: BLE001
            last_exc = e
            import time
            time.sleep(2.0 * (attempt + 1))
    else:
        raise last_exc
    global LAST_RESULTS
    LAST_RESULTS = res
    return np.concatenate(
        [np.asarray(r["out"], dtype=np.float32) for r in res.results], axis=0)
